# revision 26
# baseline (speedup 1.0000x reference)
"""Trainium2 Bass kernel for nn_GAT_87617332838818.

Mathematical collapse: the reference GAT aggregates ``alpha * hp[:, dst]``
over incoming edges per destination node.  Since the softmax weights alpha
sum to exactly 1 within each destination segment and the aggregated message
``hp[dst]`` is constant within the segment, the whole message-passing step
is the identity: ``out[n] = hp[n]``.  The network therefore reduces to a
per-node 3-layer MLP:

    logits = W2r @ elu(W1r @ elu(W0r @ x^T))        (per node column)

with W0r = W0.reshape(96,128), W1r = W1.reshape(96,96), W2r = W2.reshape(40,96)
(head-concat order matches the plain reshape).  Verified numerically against
the reference: rel fro err 4e-7 in f32; 4.5e-3 with this device pipeline.

Device strategy (8 NeuronCores, node-sharded 6250 rows each):
  - activations kept feature-on-partition: xT [128, n], h [96, n]
  - ELU via the split  elu(p') + 1 = max(p',0) + min(exp(p'),1)  with
    p' = p + nb (nb folds the "+1" inflation of the previous layer:
    nb = -W @ ones).  r = max(p+nb,0) and t = min(exp(p+nb),1) are fed
    through TWO accumulating matmuls (linearity), so the inflated h+1 is
    only ever formed in f32 PSUM — bf16-safe.
  - final layer bias cb2 = W2 @ ones subtracted in the output drain pass.
  - pipeline works on 512-column groups (one PSUM bank per matmul).  L2
    outputs of consecutive groups are packed vertically (partitions 0:40
    and 64:104 — PSUM base partitions must be 0/32/64) into one [104,512]
    PSUM tile so one drain pass and paired DMAs cover both groups.
  - PSUM drains split between DVE and ACT for engine balance (any pass
    reading f32 PSUM runs at 1x; only 16-bit SBUF passes get 2x/4x modes).
  - NOTE: engine passes whose PSUM AP spans two banks crashed the device
    (NRT_EXEC_UNIT_UNRECOVERABLE) — keep all PSUM APs within one bank.
  - 3-stage software-pipelined emission so each engine's in-order stream
    always has ready work (avoids head-of-line blocking across pairs).
  - dummy matmuls parked in the DMA-bound head flip the PE HAM clock
    gate to 2.4 GHz before the real matmuls start (measured 427->216 ns).
  - w0 rides in the first x DMA batch; w1/w2 and biases are packed into
    single DMAs to cut ~620 ns/issue sequencer serialization.
"""

import os
import sys

import numpy as np

for _p in ("/root/.axon_site/_ro/trn_rl_repo", "/opt/trn_rl_repo"):
    if os.path.isdir(_p) and _p not in sys.path:
        sys.path.append(_p)

import concourse.bass as bass
import concourse.tile as tile
from concourse import bacc, mybir
from concourse.bass_utils import run_bass_kernel_spmd

N_CORES = 8
N_PER = 6250            # 50000 / 8
D_IN = 128
D_HID = 96
D_OUT = 40
MM_N = 512              # matmul moving free-dim (1 PSUM bank)
FDP = 1024              # pair-tick free-dim (2 PSUM banks)

F16 = mybir.dt.float16
BF16 = mybir.dt.bfloat16
F32 = mybir.dt.float32

Act = mybir.ActivationFunctionType
Alu = mybir.AluOpType

_pairs = [FDP] * (N_PER // FDP)
if N_PER % FDP:
    _pairs.append(N_PER % FDP)
P = len(_pairs)
_pstarts = [sum(_pairs[:i]) for i in range(P)]

# which L0/L1 relu drains go to ACT instead of DVE (by (pair, layer))
R_DRAIN_ON_ACT = tuple((p, 0) for p in range(P) if p % 4 != 3)
OUT_DRAIN_ON_ACT = ()
X_BATCHES = [1, 3, 3]        # pairs per input DMA (first small -> fast start)
N_WARMUP_MM = 18             # dummy matmuls to flip the PE HAM to 2.4 GHz

_batch_of = {}
_b0 = 0
for _bi, _bn in enumerate(X_BATCHES):
    for _g in range(_b0, min(_b0 + _bn, P)):
        _batch_of[_g] = _bi
    _b0 += _bn
assert _b0 >= P


def _mm_splits(fd):
    """Split a pair-tick's fd into <=512 matmul chunks."""
    out = []
    j = 0
    while j < fd:
        out.append((j, min(j + MM_N, fd)))
        j += MM_N
    return out


def _build_program() -> bass.Bass:
    nc = bacc.Bacc(None, target_bir_lowering=False, debug=False)

    # xw packs [w0t | xT]: cols 0..95 = W0^T fp16, cols 96.. = x^T shard
    xw = nc.declare_dram_parameter("xw", [D_IN, D_HID + N_PER], F16,
                                   isOutput=False)
    # wb packs [w1t | w2t] bf16
    wb = nc.declare_dram_parameter("wb", [D_HID, D_HID + D_OUT], BF16,
                                   isOutput=False)
    # biases: col 0 rows 0:96 = -(W1@1); col 1 rows 0:40 & 64:104 = -(W2@1)
    bias = nc.declare_dram_parameter("bias", [104, 2], F32, isOutput=False)
    yT = nc.declare_dram_parameter("yT", [D_OUT, N_PER], F16, isOutput=True)

    st = {}
    st_batch = {}
    batch_tiles = {}

    with tile.TileContext(nc) as tc:
        with (
            tc.tile_pool(name="consts", bufs=1) as consts,
            tc.tile_pool(name="x0", bufs=1) as x0pool,
            tc.tile_pool(name="xin", bufs=2) as xpool,
            tc.tile_pool(name="sb", bufs=3) as sb,
            tc.tile_pool(name="ps0", bufs=3, space="PSUM") as ps0,
            tc.tile_pool(name="ps1", bufs=3, space="PSUM") as ps1,
            tc.tile_pool(name="ps2", bufs=2, space="PSUM") as ps2,
        ):
            # --- PE warm-up on garbage SBUF during the DMA-bound head.
            junk_w = consts.tile([D_IN, D_OUT], F16, tag="junkw")
            junk_x = consts.tile([D_IN, MM_N], F16, tag="junkx")
            nc.gpsimd.memset(junk_w[:], 0.0)
            nc.gpsimd.memset(junk_x[:], 0.0)
            warm = ps2.tile([104, MM_N], F32, tag="p2")
            for _ in range(N_WARMUP_MM):
                nc.tensor.matmul(warm[:D_OUT], junk_w[:], junk_x[:],
                                 start=True, stop=True)

            wb_sb = consts.tile([D_HID, D_HID + D_OUT], BF16, tag="wb")
            bias_sb = consts.tile([104, 2], F32, tag="bias")
            w1_sb = wb_sb[:, :D_HID]
            w2_sb = wb_sb[:, D_HID:D_HID + D_OUT]
            nb1_sb = bias_sb[:D_HID, 0:1]
            ncb2d_sb = bias_sb[:104, 1:2]

            def relu_drain(out_ap, psum_ap, bias_ap, on_act):
                """out = max(psum + bias, 0), PSUM -> SBUF bf16."""
                if on_act:
                    nc.scalar.activation(out_ap, psum_ap, Act.Relu,
                                         bias=(bias_ap if bias_ap is not None
                                               else 0.0))
                elif bias_ap is None:
                    nc.vector.tensor_scalar_max(out_ap, psum_ap, 0.0)
                else:
                    nc.vector.tensor_scalar(out_ap, psum_ap, bias_ap, 0.0,
                                            Alu.add, Alu.max)

            def exp_elu(p, lyr, psum, fd, bias_ap):
                """From psum: e=exp(p+nb), r=max(p+nb,0), t=min(e,1).

                PSUM-reading passes stay within one 512-col bank (engine
                PSUM APs must not cross banks); the SBUF-side t pass runs
                full width."""
                e = sb.tile([D_HID, FDP], BF16, tag=f"e{lyr}")
                r = sb.tile([D_HID, FDP], BF16, tag=f"r{lyr}")
                for j0, j1 in _mm_splits(fd):
                    if bias_ap is None:
                        nc.scalar.activation(e[:, j0:j1], psum[:, j0:j1],
                                             Act.Exp)
                    else:
                        nc.scalar.activation(e[:, j0:j1], psum[:, j0:j1],
                                             Act.Exp, bias=bias_ap)
                    relu_drain(r[:, j0:j1], psum[:, j0:j1], bias_ap,
                               (p, lyr) in R_DRAIN_ON_ACT)
                t = sb.tile([D_HID, FDP], BF16, tag=f"t{lyr}")
                nc.vector.tensor_scalar_min(t[:, :fd], e[:, :fd], 1.0)
                return r, t

            def stage_load(p):
                bi = _batch_of[p]
                if p > 0 and _batch_of[p - 1] == bi:
                    st[p] = st_batch[bi]
                    return
                p1_ = p
                while p1_ + 1 < P and _batch_of[p1_ + 1] == bi:
                    p1_ += 1
                lo = _pstarts[p] + (0 if bi else -D_HID)   # batch 0 incl. w0
                hi = _pstarts[p1_] + _pairs[p1_]
                cols = hi - lo
                pool = x0pool if bi == 0 else xpool
                width = D_HID + FDP * X_BATCHES[0] if bi == 0 else FDP * 4
                xt = pool.tile([D_IN, width], F16,
                               tag=("xt0" if bi == 0 else "xt"))
                nc.sync.dma_start(xt[:, :cols], xw[:, D_HID + lo:D_HID + hi])
                st_batch[bi] = {"xt": xt, "base": lo}
                st[p] = st_batch[bi]

            def stage0_mm(p):
                fd = _pairs[p]
                s = dict(st[p])
                st[p] = s
                xo = _pstarts[p] - s["base"]
                w0_sb = batch_tiles["w0"]
                p0 = ps0.tile([D_HID, FDP], F32, tag="p0")
                for j0, j1 in _mm_splits(fd):
                    nc.tensor.matmul(p0[:, j0:j1], w0_sb,
                                     s["xt"][:, xo + j0:xo + j1],
                                     start=True, stop=True)
                s["p0"] = p0

            def stage0_elu(p):
                s = st[p]
                s["r1"], s["t1"] = exp_elu(p, 0, s.pop("p0"), _pairs[p], None)

            def stage1_mm(p):
                fd = _pairs[p]
                s = st[p]
                p1 = ps1.tile([D_HID, FDP], F32, tag="p1")
                for j0, j1 in _mm_splits(fd):
                    nc.tensor.matmul(p1[:, j0:j1], w1_sb, s["r1"][:, j0:j1],
                                     start=True, stop=False)
                    nc.tensor.matmul(p1[:, j0:j1], w1_sb, s["t1"][:, j0:j1],
                                     start=False, stop=True)
                s["p1"] = p1

            def stage1_elu(p):
                s = st[p]
                s["r2"], s["t2"] = exp_elu(p, 1, s.pop("p1"), _pairs[p],
                                           nb1_sb)

            pair_state = {}

            def stage2(p):
                fd = _pairs[p]
                s = st.pop(p)
                if p % 2 == 0:
                    p2 = ps2.tile([104, FDP], F32, tag="p2")
                    pair_state[p // 2] = p2
                    rows = slice(0, D_OUT)
                else:
                    p2 = pair_state[p // 2]
                    rows = slice(64, 64 + D_OUT)
                nc.tensor.matmul(p2[rows, :fd], w2_sb, s["r2"][:, :fd],
                                 start=True, stop=False)
                nc.tensor.matmul(p2[rows, :fd], w2_sb, s["t2"][:, :fd],
                                 start=False, stop=True)
                if not ((p % 2 == 1) or (p == P - 1)):
                    return
                nrows = 104 if p % 2 == 1 else D_OUT
                o = sb.tile([104, FDP], F16, tag="o")
                if p in OUT_DRAIN_ON_ACT:
                    nc.scalar.activation(o[:nrows, :fd], p2[:nrows, :fd],
                                         Act.Identity, bias=ncb2d_sb[:nrows])
                else:
                    nc.vector.tensor_scalar_add(o[:nrows, :fd],
                                                p2[:nrows, :fd],
                                                ncb2d_sb[:nrows])
                pa = p - 1 if p % 2 == 1 else p
                fda = _pairs[pa]
                nc.gpsimd.dma_start(yT[:, _pstarts[pa]:_pstarts[pa] + fda],
                                    o[0:D_OUT, :fda])
                if p % 2 == 1:
                    nc.sync.dma_start(yT[:, _pstarts[p]:_pstarts[p] + fd],
                                      o[64:64 + D_OUT, :fd])

            # 5-deep software-pipelined emission: each engine's tick-t work
            # consumes only results finished by tick t-1, so in-order engine
            # streams never stall on same-tick producers.
            for pp in range(P + 5):
                if pp < P:
                    stage_load(pp)
                    if pp == 0:
                        batch_tiles["w0"] = st[0]["xt"][:, 0:D_HID]
                        # consts issue after the first x batch (off the
                        # critical path of the first matmul)
                        nc.sync.dma_start(wb_sb[:], wb[:])
                        nc.sync.dma_start(bias_sb[:], bias[:])
                if 0 <= pp - 1 < P:
                    stage0_mm(pp - 1)
                if 0 <= pp - 2 < P:
                    stage0_elu(pp - 2)
                if 0 <= pp - 3 < P:
                    stage1_mm(pp - 3)
                if 0 <= pp - 4 < P:
                    stage1_elu(pp - 4)
                if 0 <= pp - 5 < P:
                    stage2(pp - 5)

    nc.compile()
    return nc


_prog_cache = []
last_result = None


def kernel(**inputs) -> np.ndarray:
    global last_result
    x = np.asarray(inputs["x"], np.float32)           # [50000, 128]
    W0 = np.asarray(inputs["W0"], np.float32).reshape(D_HID, D_IN)
    W1 = np.asarray(inputs["W1"], np.float32).reshape(D_HID, D_HID)
    W2 = np.asarray(inputs["W2"], np.float32).reshape(D_OUT, D_HID)

    n = x.shape[0]
    assert n == N_CORES * N_PER, f"unexpected node count {n}"

    import ml_dtypes
    xT16 = x.T.astype(np.float16)                            # [128, 50000]
    w0t = W0.T.astype(np.float16)                            # [128, 96]
    w1tb = W1.T.astype(ml_dtypes.bfloat16)                   # [96, 96]
    w2tb = W2.T.astype(ml_dtypes.bfloat16)                   # [96, 40]
    wb = np.ascontiguousarray(
        np.concatenate([w1tb, w2tb], axis=1))                # [96, 136]
    biasm = np.zeros((104, 2), np.float32)
    biasm[:D_HID, 0] = -w1tb.astype(np.float32).sum(axis=0)  # -(W1 @ 1)
    ncb2 = -w2tb.astype(np.float32).sum(axis=0)              # -(W2 @ 1)
    biasm[:D_OUT, 1] = ncb2
    biasm[64:64 + D_OUT, 1] = ncb2                           # replicated

    if not _prog_cache:
        _prog_cache.append(_build_program())
    nc = _prog_cache[0]

    in_maps = []
    for i in range(N_CORES):
        xwi = np.ascontiguousarray(
            np.concatenate([w0t, xT16[:, i * N_PER:(i + 1) * N_PER]], axis=1))
        in_maps.append(dict(xw=xwi, wb=wb, bias=biasm))
    res = run_bass_kernel_spmd(nc, in_maps, list(range(N_CORES)))
    last_result = res
    out = np.concatenate(
        [np.asarray(res.results[i]["yT"], np.float32).T for i in range(N_CORES)],
        axis=0,
    )
    return out


if __name__ == "__main__":
    data = np.load("/tmp/gat_inputs.npz")
    y = kernel(**{k: data[k] for k in data.files})
    print("out", y.shape, y.dtype, "absmax", np.abs(y).max())


# revision 27
# speedup vs baseline: 1.1639x; 1.1639x over previous
"""Trainium2 Bass kernel for nn_GAT_87617332838818.

Mathematical collapse: the reference GAT aggregates ``alpha * hp[:, dst]``
over incoming edges per destination node.  Since the softmax weights alpha
sum to exactly 1 within each destination segment and the aggregated message
``hp[dst]`` is constant within the segment, the whole message-passing step
is the identity: ``out[n] = hp[n]``.  The network therefore reduces to a
per-node 3-layer MLP:

    logits = W2r @ elu(W1r @ elu(W0r @ x^T))        (per node column)

with W0r = W0.reshape(96,128), W1r = W1.reshape(96,96), W2r = W2.reshape(40,96)
(head-concat order matches the plain reshape).  Verified numerically against
the reference: rel fro err 4e-7 in f32; 4.5e-3 with this device pipeline.

Device strategy (8 NeuronCores, node-sharded 6250 rows each):
  - activations kept feature-on-partition: xT [128, n], h [96, n]
  - ELU via the split  elu(p') + 1 = max(p',0) + min(exp(p'),1)  with
    p' = p + nb (nb folds the "+1" inflation of the previous layer:
    nb = -W @ ones).  r = max(p+nb,0) and t = min(exp(p+nb),1) are fed
    through TWO accumulating matmuls (linearity), so the inflated h+1 is
    only ever formed in f32 PSUM — bf16-safe.
  - final layer bias cb2 = W2 @ ones subtracted in the output drain pass.
  - pipeline works on 512-column groups (one PSUM bank per matmul).  L2
    outputs of consecutive groups are packed vertically (partitions 0:40
    and 64:104 — PSUM base partitions must be 0/32/64) into one [104,512]
    PSUM tile so one drain pass and paired DMAs cover both groups.
  - PSUM drains split between DVE and ACT for engine balance (any pass
    reading f32 PSUM runs at 1x; only 16-bit SBUF passes get 2x/4x modes).
  - NOTE: engine passes whose PSUM AP spans two banks crashed the device
    (NRT_EXEC_UNIT_UNRECOVERABLE) — keep all PSUM APs within one bank.
  - 3-stage software-pipelined emission so each engine's in-order stream
    always has ready work (avoids head-of-line blocking across pairs).
  - dummy matmuls parked in the DMA-bound head flip the PE HAM clock
    gate to 2.4 GHz before the real matmuls start (measured 427->216 ns).
  - w0 rides in the first x DMA batch; w1/w2 and biases are packed into
    single DMAs to cut ~620 ns/issue sequencer serialization.
"""

import os
import sys

import numpy as np

for _p in ("/root/.axon_site/_ro/trn_rl_repo", "/opt/trn_rl_repo"):
    if os.path.isdir(_p) and _p not in sys.path:
        sys.path.append(_p)

import concourse.bass as bass
import concourse.tile as tile
from concourse import bacc, mybir
from concourse.bass_utils import run_bass_kernel_spmd

N_CORES = 8
N_PER = 6250            # 50000 / 8
D_IN = 128
D_HID = 96
D_OUT = 40
MM_N = 512              # matmul moving free-dim (1 PSUM bank)
FDP = 1024              # pair-tick free-dim (2 PSUM banks)

F16 = mybir.dt.float16
BF16 = mybir.dt.bfloat16
F32 = mybir.dt.float32

Act = mybir.ActivationFunctionType
Alu = mybir.AluOpType

_pairs = [FDP] * (N_PER // FDP)
if N_PER % FDP:
    _pairs.append(N_PER % FDP)
P = len(_pairs)
_pstarts = [sum(_pairs[:i]) for i in range(P)]

# which L0/L1 relu drains go to ACT instead of DVE (by (pair, layer))
R_DRAIN_ON_ACT = tuple((p, 0) for p in range(P) if p % 4 != 3)
OUT_DRAIN_ON_ACT = ()
X_BATCHES = [1, 3, 3]        # pairs per input DMA (first small -> fast start)
N_WARMUP_MM = 18             # dummy matmuls to flip the PE HAM to 2.4 GHz

_batch_of = {}
_b0 = 0
for _bi, _bn in enumerate(X_BATCHES):
    for _g in range(_b0, min(_b0 + _bn, P)):
        _batch_of[_g] = _bi
    _b0 += _bn
assert _b0 >= P


def _mm_splits(fd):
    """Split a pair-tick's fd into <=512 matmul chunks."""
    out = []
    j = 0
    while j < fd:
        out.append((j, min(j + MM_N, fd)))
        j += MM_N
    return out


def _build_program() -> bass.Bass:
    nc = bacc.Bacc(None, target_bir_lowering=False, debug=False)

    # xw packs [w0t | xT]: cols 0..95 = W0^T fp16, cols 96.. = x^T shard
    xw = nc.declare_dram_parameter("xw", [D_IN, D_HID + N_PER], F16,
                                   isOutput=False)
    # wb packs [w1t | w2t] bf16
    wb = nc.declare_dram_parameter("wb", [D_HID, D_HID + D_OUT], BF16,
                                   isOutput=False)
    # biases: col 0 rows 0:96 = -(W1@1); col 1 rows 0:40 & 64:104 = -(W2@1)
    bias = nc.declare_dram_parameter("bias", [104, 2], F32, isOutput=False)
    yT = nc.declare_dram_parameter("yT", [D_OUT, N_PER], F16, isOutput=True)

    st = {}
    st_batch = {}
    batch_tiles = {}

    with tile.TileContext(nc) as tc:
        with (
            tc.tile_pool(name="consts", bufs=1) as consts,
            tc.tile_pool(name="x0", bufs=1) as x0pool,
            tc.tile_pool(name="xin", bufs=2) as xpool,
            tc.tile_pool(name="sb", bufs=3) as sb,
            tc.tile_pool(name="ps0", bufs=3, space="PSUM") as ps0,
            tc.tile_pool(name="ps1", bufs=3, space="PSUM") as ps1,
            tc.tile_pool(name="ps2", bufs=2, space="PSUM") as ps2,
        ):
            # --- PE warm-up on garbage SBUF during the DMA-bound head.
            junk_w = consts.tile([D_IN, D_OUT], F16, tag="junkw")
            junk_x = consts.tile([D_IN, MM_N], F16, tag="junkx")
            nc.gpsimd.memset(junk_w[:], 0.0)
            nc.gpsimd.memset(junk_x[:], 0.0)
            warm = ps2.tile([104, MM_N], F32, tag="p2")
            for _ in range(N_WARMUP_MM):
                nc.tensor.matmul(warm[:D_OUT], junk_w[:], junk_x[:],
                                 start=True, stop=True)

            wb_sb = consts.tile([D_HID, D_HID + D_OUT], BF16, tag="wb")
            bias_sb = consts.tile([104, 2], F32, tag="bias")
            w1_sb = wb_sb[:, :D_HID]
            w2_sb = wb_sb[:, D_HID:D_HID + D_OUT]
            nb1_sb = bias_sb[:D_HID, 0:1]
            ncb2d_sb = bias_sb[:104, 1:2]

            def relu_drain(out_ap, psum_ap, bias_ap, on_act):
                """out = max(psum + bias, 0), PSUM -> SBUF bf16."""
                if on_act:
                    nc.scalar.activation(out_ap, psum_ap, Act.Relu,
                                         bias=(bias_ap if bias_ap is not None
                                               else 0.0))
                elif bias_ap is None:
                    nc.vector.tensor_scalar_max(out_ap, psum_ap, 0.0)
                else:
                    nc.vector.tensor_scalar(out_ap, psum_ap, bias_ap, 0.0,
                                            Alu.add, Alu.max)

            def exp_elu(p, lyr, psum, fd, bias_ap):
                """From psum: e=exp(p+nb), r=max(p+nb,0), t=min(e,1).

                PSUM-reading passes stay within one 512-col bank (engine
                PSUM APs must not cross banks); the SBUF-side t pass runs
                full width."""
                e = sb.tile([D_HID, FDP], BF16, tag=f"e{lyr}")
                r = sb.tile([D_HID, FDP], BF16, tag=f"r{lyr}")
                for j0, j1 in _mm_splits(fd):
                    if bias_ap is None:
                        nc.scalar.activation(e[:, j0:j1], psum[:, j0:j1],
                                             Act.Exp)
                    else:
                        nc.scalar.activation(e[:, j0:j1], psum[:, j0:j1],
                                             Act.Exp, bias=bias_ap)
                    relu_drain(r[:, j0:j1], psum[:, j0:j1], bias_ap,
                               (p, lyr) in R_DRAIN_ON_ACT)
                t = sb.tile([D_HID, FDP], BF16, tag=f"t{lyr}")
                nc.vector.tensor_scalar_min(t[:, :fd], e[:, :fd], 1.0)
                return r, t

            def stage_load(p):
                bi = _batch_of[p]
                if p > 0 and _batch_of[p - 1] == bi:
                    st[p] = st_batch[bi]
                    return
                p1_ = p
                while p1_ + 1 < P and _batch_of[p1_ + 1] == bi:
                    p1_ += 1
                lo = _pstarts[p] + (0 if bi else -D_HID)   # batch 0 incl. w0
                hi = _pstarts[p1_] + _pairs[p1_]
                cols = hi - lo
                pool = x0pool if bi == 0 else xpool
                width = D_HID + FDP * X_BATCHES[0] if bi == 0 else FDP * 4
                xt = pool.tile([D_IN, width], F16,
                               tag=("xt0" if bi == 0 else "xt"))
                nc.sync.dma_start(xt[:, :cols], xw[:, D_HID + lo:D_HID + hi])
                st_batch[bi] = {"xt": xt, "base": lo}
                st[p] = st_batch[bi]

            def stage0_mm(p):
                fd = _pairs[p]
                s = dict(st[p])
                st[p] = s
                xo = _pstarts[p] - s["base"]
                w0_sb = batch_tiles["w0"]
                p0 = ps0.tile([D_HID, FDP], F32, tag="p0")
                for j0, j1 in _mm_splits(fd):
                    nc.tensor.matmul(p0[:, j0:j1], w0_sb,
                                     s["xt"][:, xo + j0:xo + j1],
                                     start=True, stop=True)
                s["p0"] = p0

            def stage0_elu(p):
                s = st[p]
                s["r1"], s["t1"] = exp_elu(p, 0, s.pop("p0"), _pairs[p], None)

            def stage1_mm(p):
                fd = _pairs[p]
                s = st[p]
                p1 = ps1.tile([D_HID, FDP], F32, tag="p1")
                for j0, j1 in _mm_splits(fd):
                    nc.tensor.matmul(p1[:, j0:j1], w1_sb, s["r1"][:, j0:j1],
                                     start=True, stop=False)
                    nc.tensor.matmul(p1[:, j0:j1], w1_sb, s["t1"][:, j0:j1],
                                     start=False, stop=True)
                s["p1"] = p1

            def stage1_elu(p):
                s = st[p]
                s["r2"], s["t2"] = exp_elu(p, 1, s.pop("p1"), _pairs[p],
                                           nb1_sb)

            pair_state = {}

            def stage2(p):
                fd = _pairs[p]
                s = st.pop(p)
                if p % 2 == 0:
                    p2 = ps2.tile([104, FDP], F32, tag="p2")
                    pair_state[p // 2] = p2
                    rows = slice(0, D_OUT)
                else:
                    p2 = pair_state[p // 2]
                    rows = slice(64, 64 + D_OUT)
                nc.tensor.matmul(p2[rows, :fd], w2_sb, s["r2"][:, :fd],
                                 start=True, stop=False)
                nc.tensor.matmul(p2[rows, :fd], w2_sb, s["t2"][:, :fd],
                                 start=False, stop=True)
                if not ((p % 2 == 1) or (p == P - 1)):
                    return
                nrows = 104 if p % 2 == 1 else D_OUT
                o = sb.tile([104, FDP], F16, tag="o")
                if p in OUT_DRAIN_ON_ACT:
                    nc.scalar.activation(o[:nrows, :fd], p2[:nrows, :fd],
                                         Act.Identity, bias=ncb2d_sb[:nrows])
                else:
                    nc.vector.tensor_scalar_add(o[:nrows, :fd],
                                                p2[:nrows, :fd],
                                                ncb2d_sb[:nrows])
                pa = p - 1 if p % 2 == 1 else p
                fda = _pairs[pa]
                nc.gpsimd.dma_start(yT[:, _pstarts[pa]:_pstarts[pa] + fda],
                                    o[0:D_OUT, :fda])
                if p % 2 == 1:
                    nc.sync.dma_start(yT[:, _pstarts[p]:_pstarts[p] + fd],
                                      o[64:64 + D_OUT, :fd])

            # 3-deep software-pipelined emission (measured best: deeper
            # skew spreads PE work thinner, drops the HAM clock gate back
            # to 1.2 GHz and saturates the bufs=3 tile lifetimes).
            for pp in range(P + 3):
                if pp < P:
                    stage_load(pp)
                    if pp == 0:
                        batch_tiles["w0"] = st[0]["xt"][:, 0:D_HID]
                        # consts issue after the first x batch (off the
                        # critical path of the first matmul)
                        nc.sync.dma_start(wb_sb[:], wb[:])
                        nc.sync.dma_start(bias_sb[:], bias[:])
                if 0 <= pp - 1 < P:
                    stage0_mm(pp - 1)
                    stage0_elu(pp - 1)
                if 0 <= pp - 2 < P:
                    stage1_mm(pp - 2)
                    stage1_elu(pp - 2)
                if 0 <= pp - 3 < P:
                    stage2(pp - 3)

    nc.compile()
    return nc


_prog_cache = []
last_result = None


def kernel(**inputs) -> np.ndarray:
    global last_result
    x = np.asarray(inputs["x"], np.float32)           # [50000, 128]
    W0 = np.asarray(inputs["W0"], np.float32).reshape(D_HID, D_IN)
    W1 = np.asarray(inputs["W1"], np.float32).reshape(D_HID, D_HID)
    W2 = np.asarray(inputs["W2"], np.float32).reshape(D_OUT, D_HID)

    n = x.shape[0]
    assert n == N_CORES * N_PER, f"unexpected node count {n}"

    import ml_dtypes
    xT16 = x.T.astype(np.float16)                            # [128, 50000]
    w0t = W0.T.astype(np.float16)                            # [128, 96]
    w1tb = W1.T.astype(ml_dtypes.bfloat16)                   # [96, 96]
    w2tb = W2.T.astype(ml_dtypes.bfloat16)                   # [96, 40]
    wb = np.ascontiguousarray(
        np.concatenate([w1tb, w2tb], axis=1))                # [96, 136]
    biasm = np.zeros((104, 2), np.float32)
    biasm[:D_HID, 0] = -w1tb.astype(np.float32).sum(axis=0)  # -(W1 @ 1)
    ncb2 = -w2tb.astype(np.float32).sum(axis=0)              # -(W2 @ 1)
    biasm[:D_OUT, 1] = ncb2
    biasm[64:64 + D_OUT, 1] = ncb2                           # replicated

    if not _prog_cache:
        _prog_cache.append(_build_program())
    nc = _prog_cache[0]

    in_maps = []
    for i in range(N_CORES):
        xwi = np.ascontiguousarray(
            np.concatenate([w0t, xT16[:, i * N_PER:(i + 1) * N_PER]], axis=1))
        in_maps.append(dict(xw=xwi, wb=wb, bias=biasm))
    res = run_bass_kernel_spmd(nc, in_maps, list(range(N_CORES)))
    last_result = res
    out = np.concatenate(
        [np.asarray(res.results[i]["yT"], np.float32).T for i in range(N_CORES)],
        axis=0,
    )
    return out


if __name__ == "__main__":
    data = np.load("/tmp/gat_inputs.npz")
    y = kernel(**{k: data[k] for k in data.files})
    print("out", y.shape, y.dtype, "absmax", np.abs(y).max())


# revision 28
# speedup vs baseline: 1.1670x; 1.0027x over previous
"""Trainium2 Bass kernel for nn_GAT_87617332838818.

Mathematical collapse: the reference GAT aggregates ``alpha * hp[:, dst]``
over incoming edges per destination node.  Since the softmax weights alpha
sum to exactly 1 within each destination segment and the aggregated message
``hp[dst]`` is constant within the segment, the whole message-passing step
is the identity: ``out[n] = hp[n]``.  The network therefore reduces to a
per-node 3-layer MLP:

    logits = W2r @ elu(W1r @ elu(W0r @ x^T))        (per node column)

with W0r = W0.reshape(96,128), W1r = W1.reshape(96,96), W2r = W2.reshape(40,96)
(head-concat order matches the plain reshape).  Verified numerically against
the reference: rel fro err 4e-7 in f32; 4.5e-3 with this device pipeline.

Device strategy (8 NeuronCores, node-sharded 6250 rows each):
  - activations kept feature-on-partition: xT [128, n], h [96, n]
  - ELU via the split  elu(p') + 1 = max(p',0) + min(exp(p'),1)  with
    p' = p + nb (nb folds the "+1" inflation of the previous layer:
    nb = -W @ ones).  r = max(p+nb,0) and t = min(exp(p+nb),1) are fed
    through TWO accumulating matmuls (linearity), so the inflated h+1 is
    only ever formed in f32 PSUM — bf16-safe.
  - final layer bias cb2 = W2 @ ones subtracted in the output drain pass.
  - pipeline works on 512-column groups (one PSUM bank per matmul).  L2
    outputs of consecutive groups are packed vertically (partitions 0:40
    and 64:104 — PSUM base partitions must be 0/32/64) into one [104,512]
    PSUM tile so one drain pass and paired DMAs cover both groups.
  - PSUM drains split between DVE and ACT for engine balance (any pass
    reading f32 PSUM runs at 1x; only 16-bit SBUF passes get 2x/4x modes).
  - NOTE: engine passes whose PSUM AP spans two banks crashed the device
    (NRT_EXEC_UNIT_UNRECOVERABLE) — keep all PSUM APs within one bank.
  - 3-stage software-pipelined emission so each engine's in-order stream
    always has ready work (avoids head-of-line blocking across pairs).
  - dummy matmuls parked in the DMA-bound head flip the PE HAM clock
    gate to 2.4 GHz before the real matmuls start (measured 427->216 ns).
  - w0 rides in the first x DMA batch; w1/w2 and biases are packed into
    single DMAs to cut ~620 ns/issue sequencer serialization.
"""

import os
import sys

import numpy as np

for _p in ("/root/.axon_site/_ro/trn_rl_repo", "/opt/trn_rl_repo"):
    if os.path.isdir(_p) and _p not in sys.path:
        sys.path.append(_p)

import concourse.bass as bass
import concourse.tile as tile
from concourse import bacc, mybir
from concourse.bass_utils import run_bass_kernel_spmd

N_CORES = 8
N_PER = 6250            # 50000 / 8
D_IN = 128
D_HID = 96
D_OUT = 40
MM_N = 512              # matmul moving free-dim (1 PSUM bank)
FDP = 1024              # pair-tick free-dim (2 PSUM banks)

F16 = mybir.dt.float16
BF16 = mybir.dt.bfloat16
F32 = mybir.dt.float32

Act = mybir.ActivationFunctionType
Alu = mybir.AluOpType

_pairs = [FDP] * (N_PER // FDP)
if N_PER % FDP:
    _pairs.append(N_PER % FDP)
P = len(_pairs)
_pstarts = [sum(_pairs[:i]) for i in range(P)]

# which L0/L1 relu drains go to ACT instead of DVE (by (pair, layer))
R_DRAIN_ON_ACT = tuple((p, 0) for p in range(P) if p % 4 != 3)
OUT_DRAIN_ON_ACT = ()
X_BATCHES = [1, 3, 3]        # pairs per input DMA (first small -> fast start)
N_WARMUP_MM = 18             # dummy matmuls to flip the PE HAM to 2.4 GHz

_batch_of = {}
_b0 = 0
for _bi, _bn in enumerate(X_BATCHES):
    for _g in range(_b0, min(_b0 + _bn, P)):
        _batch_of[_g] = _bi
    _b0 += _bn
assert _b0 >= P


def _mm_splits(fd):
    """Split a pair-tick's fd into <=512 matmul chunks."""
    out = []
    j = 0
    while j < fd:
        out.append((j, min(j + MM_N, fd)))
        j += MM_N
    return out


def _build_program() -> bass.Bass:
    nc = bacc.Bacc(None, target_bir_lowering=False, debug=False)

    # xw packs [w0t | xT]: cols 0..95 = W0^T fp16, cols 96.. = x^T shard
    xw = nc.declare_dram_parameter("xw", [D_IN, D_HID + N_PER], F16,
                                   isOutput=False)
    # wb packs [w1t | w2t] bf16
    wb = nc.declare_dram_parameter("wb", [D_HID, D_HID + D_OUT], BF16,
                                   isOutput=False)
    # biases: col 0 rows 0:96 = -(W1@1); col 1 rows 0:40 & 64:104 = -(W2@1)
    bias = nc.declare_dram_parameter("bias", [104, 2], F32, isOutput=False)
    # packed output: pair k at cols [512k, 512k+512): rows 0:40 = group 2k,
    # rows 64:104 = group 2k+1 (rows 40:64 unused). Host unpacks.
    yT = nc.declare_dram_parameter("yT", [104, 3178], F16, isOutput=True)

    st = {}
    st_batch = {}
    batch_tiles = {}

    with tile.TileContext(nc) as tc:
        with (
            tc.tile_pool(name="consts", bufs=1) as consts,
            tc.tile_pool(name="x0", bufs=1) as x0pool,
            tc.tile_pool(name="xin", bufs=2) as xpool,
            tc.tile_pool(name="sb", bufs=3) as sb,
            tc.tile_pool(name="ps0", bufs=3, space="PSUM") as ps0,
            tc.tile_pool(name="ps1", bufs=3, space="PSUM") as ps1,
            tc.tile_pool(name="ps2", bufs=2, space="PSUM") as ps2,
        ):
            # --- PE warm-up on garbage SBUF during the DMA-bound head.
            junk_w = consts.tile([D_IN, D_OUT], F16, tag="junkw")
            junk_x = consts.tile([D_IN, MM_N], F16, tag="junkx")
            nc.gpsimd.memset(junk_w[:], 0.0)
            nc.gpsimd.memset(junk_x[:], 0.0)
            warm = ps2.tile([104, MM_N], F32, tag="p2")
            for _ in range(N_WARMUP_MM):
                nc.tensor.matmul(warm[:D_OUT], junk_w[:], junk_x[:],
                                 start=True, stop=True)

            wb_sb = consts.tile([D_HID, D_HID + D_OUT], BF16, tag="wb")
            bias_sb = consts.tile([104, 2], F32, tag="bias")
            w1_sb = wb_sb[:, :D_HID]
            w2_sb = wb_sb[:, D_HID:D_HID + D_OUT]
            nb1_sb = bias_sb[:D_HID, 0:1]
            ncb2d_sb = bias_sb[:104, 1:2]

            def relu_drain(out_ap, psum_ap, bias_ap, on_act):
                """out = max(psum + bias, 0), PSUM -> SBUF bf16."""
                if on_act:
                    nc.scalar.activation(out_ap, psum_ap, Act.Relu,
                                         bias=(bias_ap if bias_ap is not None
                                               else 0.0))
                elif bias_ap is None:
                    nc.vector.tensor_scalar_max(out_ap, psum_ap, 0.0)
                else:
                    nc.vector.tensor_scalar(out_ap, psum_ap, bias_ap, 0.0,
                                            Alu.add, Alu.max)

            def exp_elu(p, lyr, psum, fd, bias_ap):
                """From psum: e=exp(p+nb), r=max(p+nb,0), t=min(e,1).

                PSUM-reading passes stay within one 512-col bank (engine
                PSUM APs must not cross banks); the SBUF-side t pass runs
                full width."""
                e = sb.tile([D_HID, FDP], BF16, tag=f"e{lyr}")
                r = sb.tile([D_HID, FDP], BF16, tag=f"r{lyr}")
                for j0, j1 in _mm_splits(fd):
                    if bias_ap is None:
                        nc.scalar.activation(e[:, j0:j1], psum[:, j0:j1],
                                             Act.Exp)
                    else:
                        nc.scalar.activation(e[:, j0:j1], psum[:, j0:j1],
                                             Act.Exp, bias=bias_ap)
                    relu_drain(r[:, j0:j1], psum[:, j0:j1], bias_ap,
                               (p, lyr) in R_DRAIN_ON_ACT)
                t = sb.tile([D_HID, FDP], BF16, tag=f"t{lyr}")
                nc.vector.tensor_scalar_min(t[:, :fd], e[:, :fd], 1.0)
                return r, t

            def stage_load(p):
                bi = _batch_of[p]
                if p > 0 and _batch_of[p - 1] == bi:
                    st[p] = st_batch[bi]
                    return
                p1_ = p
                while p1_ + 1 < P and _batch_of[p1_ + 1] == bi:
                    p1_ += 1
                lo = _pstarts[p] + (0 if bi else -D_HID)   # batch 0 incl. w0
                hi = _pstarts[p1_] + _pairs[p1_]
                cols = hi - lo
                pool = x0pool if bi == 0 else xpool
                width = D_HID + FDP * X_BATCHES[0] if bi == 0 else FDP * 4
                xt = pool.tile([D_IN, width], F16,
                               tag=("xt0" if bi == 0 else "xt"))
                nc.sync.dma_start(xt[:, :cols], xw[:, D_HID + lo:D_HID + hi])
                st_batch[bi] = {"xt": xt, "base": lo}
                st[p] = st_batch[bi]

            def stage0_mm(p):
                fd = _pairs[p]
                s = dict(st[p])
                st[p] = s
                xo = _pstarts[p] - s["base"]
                w0_sb = batch_tiles["w0"]
                p0 = ps0.tile([D_HID, FDP], F32, tag="p0")
                for j0, j1 in _mm_splits(fd):
                    nc.tensor.matmul(p0[:, j0:j1], w0_sb,
                                     s["xt"][:, xo + j0:xo + j1],
                                     start=True, stop=True)
                s["p0"] = p0

            def stage0_elu(p):
                s = st[p]
                s["r1"], s["t1"] = exp_elu(p, 0, s.pop("p0"), _pairs[p], None)

            def stage1_mm(p):
                fd = _pairs[p]
                s = st[p]
                p1 = ps1.tile([D_HID, FDP], F32, tag="p1")
                for j0, j1 in _mm_splits(fd):
                    nc.tensor.matmul(p1[:, j0:j1], w1_sb, s["r1"][:, j0:j1],
                                     start=True, stop=False)
                    nc.tensor.matmul(p1[:, j0:j1], w1_sb, s["t1"][:, j0:j1],
                                     start=False, stop=True)
                s["p1"] = p1

            def stage1_elu(p):
                s = st[p]
                s["r2"], s["t2"] = exp_elu(p, 1, s.pop("p1"), _pairs[p],
                                           nb1_sb)

            pair_state = {}

            def stage2(p):
                fd = _pairs[p]
                s = st.pop(p)
                if p % 2 == 0:
                    p2 = ps2.tile([104, FDP], F32, tag="p2")
                    pair_state[p // 2] = p2
                    rows = slice(0, D_OUT)
                else:
                    p2 = pair_state[p // 2]
                    rows = slice(64, 64 + D_OUT)
                nc.tensor.matmul(p2[rows, :fd], w2_sb, s["r2"][:, :fd],
                                 start=True, stop=False)
                nc.tensor.matmul(p2[rows, :fd], w2_sb, s["t2"][:, :fd],
                                 start=False, stop=True)
                if not ((p % 2 == 1) or (p == P - 1)):
                    return
                nrows = 104 if p % 2 == 1 else D_OUT
                o = sb.tile([104, FDP], F16, tag="o")
                if p in OUT_DRAIN_ON_ACT:
                    nc.scalar.activation(o[:nrows, :fd], p2[:nrows, :fd],
                                         Act.Identity, bias=ncb2d_sb[:nrows])
                else:
                    nc.vector.tensor_scalar_add(o[:nrows, :fd],
                                                p2[:nrows, :fd],
                                                ncb2d_sb[:nrows])
                kp = p // 2
                ow = fd if p % 2 == 1 else _pairs[p]
                eng = nc.gpsimd if kp % 2 == 0 else nc.sync
                eng.dma_start(yT[:, kp * FDP:kp * FDP + ow], o[:, :ow])

            # 3-deep software-pipelined emission (measured best: deeper
            # skew spreads PE work thinner, drops the HAM clock gate back
            # to 1.2 GHz and saturates the bufs=3 tile lifetimes).
            for pp in range(P + 3):
                if pp < P:
                    stage_load(pp)
                    if pp == 0:
                        batch_tiles["w0"] = st[0]["xt"][:, 0:D_HID]
                        # consts issue after the first x batch (off the
                        # critical path of the first matmul)
                        nc.sync.dma_start(wb_sb[:], wb[:])
                        nc.sync.dma_start(bias_sb[:], bias[:])
                if 0 <= pp - 1 < P:
                    stage0_mm(pp - 1)
                    stage0_elu(pp - 1)
                if 0 <= pp - 2 < P:
                    stage1_mm(pp - 2)
                    stage1_elu(pp - 2)
                if 0 <= pp - 3 < P:
                    stage2(pp - 3)

    nc.compile()
    return nc


_prog_cache = []
last_result = None


def kernel(**inputs) -> np.ndarray:
    global last_result
    x = np.asarray(inputs["x"], np.float32)           # [50000, 128]
    W0 = np.asarray(inputs["W0"], np.float32).reshape(D_HID, D_IN)
    W1 = np.asarray(inputs["W1"], np.float32).reshape(D_HID, D_HID)
    W2 = np.asarray(inputs["W2"], np.float32).reshape(D_OUT, D_HID)

    n = x.shape[0]
    assert n == N_CORES * N_PER, f"unexpected node count {n}"

    import ml_dtypes
    xT16 = x.T.astype(np.float16)                            # [128, 50000]
    w0t = W0.T.astype(np.float16)                            # [128, 96]
    w1tb = W1.T.astype(ml_dtypes.bfloat16)                   # [96, 96]
    w2tb = W2.T.astype(ml_dtypes.bfloat16)                   # [96, 40]
    wb = np.ascontiguousarray(
        np.concatenate([w1tb, w2tb], axis=1))                # [96, 136]
    biasm = np.zeros((104, 2), np.float32)
    biasm[:D_HID, 0] = -w1tb.astype(np.float32).sum(axis=0)  # -(W1 @ 1)
    ncb2 = -w2tb.astype(np.float32).sum(axis=0)              # -(W2 @ 1)
    biasm[:D_OUT, 1] = ncb2
    biasm[64:64 + D_OUT, 1] = ncb2                           # replicated

    if not _prog_cache:
        _prog_cache.append(_build_program())
    nc = _prog_cache[0]

    in_maps = []
    for i in range(N_CORES):
        xwi = np.ascontiguousarray(
            np.concatenate([w0t, xT16[:, i * N_PER:(i + 1) * N_PER]], axis=1))
        in_maps.append(dict(xw=xwi, wb=wb, bias=biasm))
    res = run_bass_kernel_spmd(nc, in_maps, list(range(N_CORES)))
    last_result = res
    out = np.empty((n, D_OUT), np.float32)
    for i in range(N_CORES):
        yt = np.asarray(res.results[i]["yT"], np.float32)  # [104, 3178]
        base = i * N_PER
        for kp in range((P + 1) // 2):
            c0 = kp * FDP
            g0 = 2 * kp
            w0_ = _pairs[g0]
            out[base + _pstarts[g0]:base + _pstarts[g0] + w0_] = \
                yt[0:D_OUT, c0:c0 + w0_].T
            if g0 + 1 < P:
                w1_ = _pairs[g0 + 1]
                out[base + _pstarts[g0 + 1]:base + _pstarts[g0 + 1] + w1_] = \
                    yt[64:64 + D_OUT, c0:c0 + w1_].T
    return out


if __name__ == "__main__":
    data = np.load("/tmp/gat_inputs.npz")
    y = kernel(**{k: data[k] for k in data.files})
    print("out", y.shape, y.dtype, "absmax", np.abs(y).max())
